# revision 8
# baseline (speedup 1.0000x reference)
"""Trainium2 Bass kernel for nn_Causal2X (GNN message passing + edge MLP +
global softmax + per-graph argmax).

Strategy (8 NeuronCores, SPMD):
  - Shard by GRAPH: core c owns graphs [64c, 64c+64). Host groups edges into
    per-(graph, dst-range) segments of fixed size, so each core processes
    Ec = 64*S edges and all segment reductions are core-local.
  - Node-row gathers use the Ant dma_gather (int16 indices):
      * src rows are graph-local (sorted batch) -> a per-core 16384-row table
        slice with local indices.
      * dst rows are random over 102400 nodes -> 4 dst-range runs
        (25600 rows each, fits int16); every 512-edge macrotile lies in
        exactly one range run, so each tile's dst gather reads one table
        slice with local indices.
  - Per 512-edge macrotile: gather src/dst rows (edge-major), PE-transpose to
    feature-major, 5-layer MLP on the PE in fp32 (required: per-graph top-2
    score gaps go down to 1e-5; bf16/f32r flip the argmax), exact ELU via
    ELU(t)+1 = min(exp(t+b), max(t+b+1, 1)) (2 DVE ops + 1 ACT op, bias
    fused, the (g-1) correction folded into the next layer's bias).
  - Per-edge label selection: y[batch[src]] is constant per graph, so
    phase 2 pulls score rows straight out of the [10, Ec] logits buffer with
    4x64-descriptor indirect DMAs (offset = y[g]*Ec + run_base + g*S_r).
  - Global softmax: exp without max-subtraction (scores are O(1)), one scalar
    AllReduce(add) for the denominator.
  - probs / seg_max / actions computed in [64, S] layout; host scatters probs
    back to original edge order.
"""

import numpy as np
from contextlib import ExitStack

import concourse.bass as bass
import concourse.tile as tile
from concourse import bacc, mybir
from concourse.masks import make_identity

F32 = mybir.dt.float32
I32 = mybir.dt.int32
I16 = mybir.dt.int16
AF = mybir.ActivationFunctionType
OP = mybir.AluOpType

NCORES = 8
N_NODES = 102400
H = 128
B_GRAPHS = 512
L = 10
G = B_GRAPHS // NCORES      # 64 graphs per core
TILE = 512                  # edges per macrotile
CHUNKS = TILE // 128
NRANGE = 4
RANGE = N_NODES // NRANGE   # 25600 rows per dst range (int16-addressable)
SLOC = 16384                # per-core src table slice rows
BIGC = float(2 ** 20)       # argmin-index offset (exact in fp32)

_NC_CACHE = {}
_LAST = {}


def _elu(nc, apool, psum_ap, be, by, gout_ap, mm_dt):
    """gout = ELU(psum + bias) + 1 == min(exp(t), max(t+1, 1)), t = psum+bias."""
    e = apool.tile([128, TILE], mm_dt, tag="elu_e", name="elu_e")
    nc.scalar.activation(e[:], psum_ap, AF.Exp, bias=be)
    yv = apool.tile([128, TILE], mm_dt, tag="elu_y", name="elu_y")
    nc.vector.tensor_scalar(yv[:], psum_ap, by, 1.0, op0=OP.add, op1=OP.max)
    nc.vector.tensor_tensor(out=gout_ap, in0=e[:], in1=yv[:], op=OP.min)


def _build_nc(Ec, S_list, mm_dt=F32):
    S = sum(S_list)
    assert Ec == G * S
    T = Ec // TILE
    # range run of each macrotile (runs are multiples of 512 slots)
    runof = []
    for t in range(T):
        s0 = t * TILE
        acc = 0
        for r in range(NRANGE):
            acc += G * S_list[r]
            if s0 < acc:
                runof.append(r)
                break
    col_base = np.concatenate([[0], np.cumsum(S_list)]).astype(int)

    nc = bacc.Bacc(
        "TRN2", target_bir_lowering=False, debug=False, num_devices=NCORES
    )

    # ---------------- I/O ----------------
    nodes_d = nc.declare_dram_parameter("nodes", [N_NODES, H], mm_dt, isOutput=False)
    snodes_d = nc.declare_dram_parameter("snodes", [SLOC, H], mm_dt, isOutput=False)
    gsrc_d = nc.declare_dram_parameter("gsrc16", [T, 128, TILE // 16], I16,
                                       isOutput=False)
    gdst_d = nc.declare_dram_parameter("gdst16", [T, 128, TILE // 16], I16,
                                       isOutput=False)
    w1_d = nc.declare_dram_parameter("w1c", [2, 4, 128, 128], mm_dt, isOutput=False)
    w2_d = nc.declare_dram_parameter("w2c", [4, 2, 128, 128], mm_dt, isOutput=False)
    w3_d = nc.declare_dram_parameter("w3c", [2, 128, 128], mm_dt, isOutput=False)
    w4_d = nc.declare_dram_parameter("w4c", [128, 128], mm_dt, isOutput=False)
    w5_d = nc.declare_dram_parameter("w5c", [128, L], mm_dt, isOutput=False)
    b1e_d = nc.declare_dram_parameter("b1e", [4, 128], F32, isOutput=False)
    b1y_d = nc.declare_dram_parameter("b1y", [4, 128], F32, isOutput=False)
    b2e_d = nc.declare_dram_parameter("b2e", [2, 128], F32, isOutput=False)
    b2y_d = nc.declare_dram_parameter("b2y", [2, 128], F32, isOutput=False)
    b4e_d = nc.declare_dram_parameter("b4e", [1, 128], F32, isOutput=False)
    b4y_d = nc.declare_dram_parameter("b4y", [1, 128], F32, isOutput=False)
    maskb_d = nc.declare_dram_parameter("maskb", [G, S], F32, isOutput=False)
    idxm_d = nc.declare_dram_parameter("idxm", [G, S], F32, isOutput=False)
    yoff_d = nc.declare_dram_parameter("yoff4", [NRANGE, G, 1], I32, isOutput=False)

    probs_o = nc.declare_dram_parameter("probs", [G, S], F32, isOutput=True)
    segmax_o = nc.declare_dram_parameter("segmax", [G, 1], F32, isOutput=True)
    act_o = nc.declare_dram_parameter("act", [G, 1], F32, isOutput=True)

    # internal DRAM
    logits_d = nc.dram_tensor("logits_i", [L, Ec], F32)
    ccin_d = nc.dram_tensor("ccin_i", [1, 1], F32)
    ccout_d = nc.dram_tensor("ccout_i", [1, 1], F32)

    dma_sem = nc.alloc_semaphore("cc_dma")
    cc_sem = nc.alloc_semaphore("cc_done")

    with tile.TileContext(nc) as tc:
        with ExitStack() as ctx0:
            wpool = ctx0.enter_context(tc.tile_pool(name="weights", bufs=1))

            ident = wpool.tile([128, 128], mm_dt, tag="ident")
            make_identity(nc, ident[:])

            w1 = [[wpool.tile([128, 128], mm_dt, tag=f"w1_{k}_{m}",
                              name=f"w1_{k}_{m}")
                   for m in range(4)] for k in range(2)]
            for k in range(2):
                for m in range(4):
                    nc.sync.dma_start(w1[k][m][:], w1_d[k, m])
            w2 = [[wpool.tile([128, 128], mm_dt, tag=f"w2_{k}_{m}",
                              name=f"w2_{k}_{m}")
                   for m in range(2)] for k in range(4)]
            for k in range(4):
                for m in range(2):
                    nc.sync.dma_start(w2[k][m][:], w2_d[k, m])
            w3 = [wpool.tile([128, 128], mm_dt, tag=f"w3_{k}", name=f"w3_{k}")
                  for k in range(2)]
            for k in range(2):
                nc.sync.dma_start(w3[k][:], w3_d[k])
            w4 = wpool.tile([128, 128], mm_dt, tag="w4")
            nc.sync.dma_start(w4[:], w4_d[:, :])
            w5 = wpool.tile([128, L], mm_dt, tag="w5")
            nc.sync.dma_start(w5[:], w5_d[:, :])

            def bias_tiles(dram, n, nm):
                ts = [wpool.tile([128, 1], F32, tag=f"{nm}_{i}",
                                 name=f"{nm}_{i}") for i in range(n)]
                for i in range(n):
                    nc.sync.dma_start(ts[i][:], dram[i, :, None])
                return ts

            b1e = bias_tiles(b1e_d, 4, "b1e")
            b1y = bias_tiles(b1y_d, 4, "b1y")
            b2e = bias_tiles(b2e_d, 2, "b2e")
            b2y = bias_tiles(b2y_d, 2, "b2y")
            b4e = bias_tiles(b4e_d, 1, "b4e")
            b4y = bias_tiles(b4y_d, 1, "b4y")

            ones_g1 = wpool.tile([G, 1], F32, tag="ones_g1")
            nc.vector.memset(ones_g1[:], 1.0)
            ones_1g = wpool.tile([1, G], F32, tag="ones_1g")
            nc.vector.memset(ones_1g[:], 1.0)

            # ---------------- phase 1: edge MLP ----------------
            with ExitStack() as ctx1:
                ipool = ctx1.enter_context(tc.tile_pool(name="idx", bufs=4))
                gpool = ctx1.enter_context(tc.tile_pool(name="gath", bufs=3))
                spool = ctx1.enter_context(tc.tile_pool(name="reps", bufs=2))
                apool = ctx1.enter_context(tc.tile_pool(name="acts", bufs=2))
                lpool = ctx1.enter_context(tc.tile_pool(name="l5", bufs=3))
                tppool = ctx1.enter_context(
                    tc.tile_pool(name="tp_ps", bufs=2, space="PSUM"))
                p1pool = ctx1.enter_context(
                    tc.tile_pool(name="p1_ps", bufs=2, space="PSUM"))
                p2pool = ctx1.enter_context(
                    tc.tile_pool(name="p2_ps", bufs=2, space="PSUM"))
                p3pool = ctx1.enter_context(
                    tc.tile_pool(name="p3_ps", bufs=2, space="PSUM"))

                for t in range(T):
                    r = runof[t]
                    idxs = ipool.tile([128, TILE // 16], I16, tag="idxs")
                    nc.sync.dma_start(idxs[:], gsrc_d[t])
                    idxd = ipool.tile([128, TILE // 16], I16, tag="idxd")
                    nc.sync.dma_start(idxd[:], gdst_d[t])

                    graw_s = gpool.tile([128, CHUNKS, H], mm_dt, tag="graw_s")
                    nc.gpsimd.dma_gather(
                        out_ap=graw_s[:], in_ap=snodes_d[:, :],
                        idxs_ap=idxs[:], num_idxs=TILE, num_idxs_reg=TILE,
                        elem_size=H)
                    graw_d = gpool.tile([128, CHUNKS, H], mm_dt, tag="graw_d")
                    nc.gpsimd.dma_gather(
                        out_ap=graw_d[:],
                        in_ap=nodes_d[r * RANGE:(r + 1) * RANGE, :],
                        idxs_ap=idxd[:], num_idxs=TILE, num_idxs_reg=TILE,
                        elem_size=H)

                    sT = spool.tile([128, TILE], mm_dt, tag="sT")
                    dT = spool.tile([128, TILE], mm_dt, tag="dT")
                    for c in range(CHUNKS):
                        tp = tppool.tile([128, 128], F32, tag="tp")
                        nc.tensor.transpose(tp[:], graw_s[:, c, :], ident[:])
                        if c % 2 == 0:
                            nc.vector.tensor_copy(
                                sT[:, c * 128:(c + 1) * 128], tp[:])
                        else:
                            nc.scalar.copy(sT[:, c * 128:(c + 1) * 128], tp[:])
                        tp2 = tppool.tile([128, 128], F32, tag="tp")
                        nc.tensor.transpose(tp2[:], graw_d[:, c, :], ident[:])
                        if c % 2 == 0:
                            nc.scalar.copy(dT[:, c * 128:(c + 1) * 128], tp2[:])
                        else:
                            nc.vector.tensor_copy(
                                dT[:, c * 128:(c + 1) * 128], tp2[:])

                    # L1: [256 -> 512]
                    g1 = apool.tile([128, 4 * TILE], mm_dt, tag="g1")
                    for m in range(4):
                        p1 = p1pool.tile([128, TILE], F32, tag="p1")
                        nc.tensor.matmul(p1[:], w1[0][m][:], sT[:],
                                         start=True, stop=False)
                        nc.tensor.matmul(p1[:], w1[1][m][:], dT[:],
                                         start=False, stop=True)
                        _elu(nc, apool, p1[:], b1e[m][:], b1y[m][:],
                             g1[:, m * TILE:(m + 1) * TILE], mm_dt)
                    # L2: [512 -> 256]
                    g2 = apool.tile([128, 2 * TILE], mm_dt, tag="g2")
                    for m in range(2):
                        p2 = p2pool.tile([128, TILE], F32, tag="p2")
                        for k in range(4):
                            nc.tensor.matmul(p2[:], w2[k][m][:],
                                             g1[:, k * TILE:(k + 1) * TILE],
                                             start=(k == 0), stop=(k == 3))
                        _elu(nc, apool, p2[:], b2e[m][:], b2y[m][:],
                             g2[:, m * TILE:(m + 1) * TILE], mm_dt)
                    # L3: [256 -> 128], pure; bias folded into L4's bias
                    p3 = p3pool.tile([128, TILE], F32, tag="p3")
                    for k in range(2):
                        nc.tensor.matmul(p3[:], w3[k][:],
                                         g2[:, k * TILE:(k + 1) * TILE],
                                         start=(k == 0), stop=(k == 1))
                    c3 = apool.tile([128, TILE], mm_dt, tag="c3")
                    nc.scalar.copy(c3[:], p3[:])
                    # L4: [128 -> 128] + ELU
                    p4 = p3pool.tile([128, TILE], F32, tag="p3")
                    nc.tensor.matmul(p4[:], w4[:], c3[:], start=True, stop=True)
                    g4 = apool.tile([128, TILE], mm_dt, tag="g4")
                    _elu(nc, apool, p4[:], b4e[0][:], b4y[0][:], g4[:], mm_dt)
                    # L5: [128 -> 10]; bias b5' folded into phase-2 mask
                    p5 = tppool.tile([L, TILE], F32, tag="tp")
                    nc.tensor.matmul(p5[:], w5[:], g4[:], start=True, stop=True)
                    l5 = lpool.tile([L, TILE], F32, tag="l5")
                    nc.vector.tensor_copy(l5[:], p5[:])
                    nc.sync.dma_start(logits_d[:, t * TILE:(t + 1) * TILE], l5[:])

            # ---------------- phase 2: softmax + segment max ----------------
            with ExitStack() as ctx2:
                qpool = ctx2.enter_context(tc.tile_pool(name="ph2", bufs=1))
                cpool = ctx2.enter_context(tc.tile_pool(name="ph2c", bufs=1))
                pspool = ctx2.enter_context(
                    tc.tile_pool(name="ph2_ps", bufs=1, space="PSUM"))

                logits_flat = logits_d.ap().rearrange("a b -> (a b)")[:, None]
                scores = qpool.tile([G, S], F32, tag="scores")
                for r in range(NRANGE):
                    yofft = cpool.tile([G, 1], I32, tag=f"yoff_{r}",
                                       name=f"yoff_{r}")
                    nc.sync.dma_start(yofft[:], yoff_d[r])
                    nc.gpsimd.indirect_dma_start(
                        out=scores[:, int(col_base[r]):int(col_base[r + 1])],
                        out_offset=None,
                        in_=logits_flat,
                        in_offset=bass.IndirectOffsetOnAxis(
                            ap=yofft[:, :1], axis=0))

                mb = qpool.tile([G, S], F32, tag="mb")
                nc.sync.dma_start(mb[:], maskb_d[:, :])
                sc2 = qpool.tile([G, S], F32, tag="sc2")
                nc.vector.tensor_tensor(out=sc2[:], in0=scores[:], in1=mb[:],
                                        op=OP.add)
                ex = qpool.tile([G, S], F32, tag="ex")
                nc.scalar.activation(ex[:], sc2[:], AF.Exp)

                rowsum = cpool.tile([G, 1], F32, tag="rowsum")
                nc.vector.tensor_reduce(rowsum[:], ex[:],
                                        axis=mybir.AxisListType.X, op=OP.add)
                rowmax = cpool.tile([G, 1], F32, tag="rowmax")
                nc.vector.tensor_reduce(rowmax[:], ex[:],
                                        axis=mybir.AxisListType.X, op=OP.max)

                gsum_ps = pspool.tile([1, 1], F32, tag="gsum_ps")
                nc.tensor.matmul(gsum_ps[:], ones_g1[:], rowsum[:],
                                 start=True, stop=True)
                gsum = cpool.tile([1, 1], F32, tag="gsum")
                nc.vector.tensor_copy(gsum[:], gsum_ps[:])

                gall = cpool.tile([1, 1], F32, tag="gall")
                with tc.tile_critical():
                    nc.gpsimd.dma_start(
                        out=ccin_d[:, :], in_=gsum[:]).then_inc(dma_sem, 16)
                    nc.gpsimd.wait_ge(dma_sem, 16)
                    nc.gpsimd.collective_compute(
                        "AllReduce", OP.add,
                        replica_groups=[list(range(NCORES))],
                        ins=[ccin_d.ap()], outs=[ccout_d.ap()],
                    ).then_inc(cc_sem, 1)
                    nc.gpsimd.wait_ge(cc_sem, 1)
                    nc.gpsimd.dma_start(
                        out=gall[:], in_=ccout_d[:, :]).then_inc(dma_sem, 16)
                    nc.gpsimd.wait_ge(dma_sem, 32)

                recip = cpool.tile([1, 1], F32, tag="recip")
                nc.vector.reciprocal(recip[:], gall[:])
                rb_ps = pspool.tile([G, 1], F32, tag="rb_ps")
                nc.tensor.matmul(rb_ps[:], ones_1g[:], recip[:],
                                 start=True, stop=True)
                rb = cpool.tile([G, 1], F32, tag="rb")
                nc.vector.tensor_copy(rb[:], rb_ps[:])

                probs_t = qpool.tile([G, S], F32, tag="probs")
                nc.vector.tensor_scalar(probs_t[:], ex[:], rb[:, :1], None,
                                        op0=OP.mult)
                nc.sync.dma_start(probs_o[:, :], probs_t[:])

                smx = cpool.tile([G, 1], F32, tag="smx")
                nc.vector.tensor_scalar(smx[:], rowmax[:], rb[:, :1], None,
                                        op0=OP.mult)
                nc.sync.dma_start(segmax_o[:, :], smx[:])

                ismax = qpool.tile([G, S], F32, tag="ismax")
                nc.vector.tensor_scalar(ismax[:], probs_t[:], smx[:, :1], None,
                                        op0=OP.is_equal)
                idxmt = qpool.tile([G, S], F32, tag="idxmt")
                nc.sync.dma_start(idxmt[:], idxm_d[:, :])
                cand = qpool.tile([G, S], F32, tag="cand")
                nc.vector.tensor_tensor(out=cand[:], in0=ismax[:], in1=idxmt[:],
                                        op=OP.mult)
                cmin = cpool.tile([G, 1], F32, tag="cmin")
                nc.vector.tensor_reduce(cmin[:], cand[:],
                                        axis=mybir.AxisListType.X, op=OP.min)
                actt = cpool.tile([G, 1], F32, tag="actt")
                nc.vector.tensor_scalar(actt[:], cmin[:], BIGC, None, op0=OP.add)
                nc.sync.dma_start(act_o[:, :], actt[:])

    nc.compile()
    return nc


def _wrap16(a):
    """[T, n] int -> dma_gather idx layout [T, 128, n/16] (16-wrap, x8)."""
    Tn, n = a.shape
    w = a.reshape(Tn, n // 16, 16).transpose(0, 2, 1)  # [T, 16, n/16]
    return np.ascontiguousarray(np.tile(w, (1, 8, 1)).astype(np.int16))


def _host_prep(node_reps, edge_index, batch, y,
               W1, b1, W2, b2, W3, b3, W4, b4, W5, b5, np_mm=np.float32):
    src = np.asarray(edge_index[0], dtype=np.int64)
    dst = np.asarray(edge_index[1], dtype=np.int64)
    batch = np.asarray(batch, dtype=np.int64)
    y = np.asarray(y, dtype=np.int64)
    E = src.shape[0]
    eb = batch[src]
    rng_e = dst // RANGE

    # per-(graph, range) segment sizes
    key = eb * NRANGE + rng_e
    cnt2 = np.bincount(key, minlength=B_GRAPHS * NRANGE).reshape(
        B_GRAPHS, NRANGE)
    S_list = [int(-(-cnt2[:, r].max() // 8) * 8) for r in range(NRANGE)]
    S_list = [max(s, 8) for s in S_list]
    S = sum(S_list)
    Ec = G * S
    run_base = np.concatenate(
        [[0], np.cumsum([G * s for s in S_list])]).astype(np.int64)
    col_base = np.concatenate([[0], np.cumsum(S_list)]).astype(np.int64)
    S_arr = np.array(S_list, np.int64)

    order = np.argsort(key, kind="stable")
    starts = np.zeros(B_GRAPHS * NRANGE + 1, np.int64)
    np.cumsum(cnt2.reshape(-1), out=starts[1:])
    j_within = np.arange(E, dtype=np.int64) - np.repeat(
        starts[:-1], cnt2.reshape(-1))
    e_sorted = order
    g_s = eb[e_sorted]
    r_s = rng_e[e_sorted]
    core_s = g_s // G
    gl_s = g_s % G

    # slot order (gather/logits space) and phase-2 position
    slot_g = core_s * Ec + run_base[r_s] + gl_s * S_arr[r_s] + j_within
    p2_g = core_s * (G * S) + gl_s * S + col_base[r_s] + j_within

    srcA = np.zeros(NCORES * Ec, np.int64)          # global src node per slot
    dstL = np.zeros(NCORES * Ec, np.int64)          # local dst idx per slot
    srcA[slot_g] = src[e_sorted]
    dstL[slot_g] = dst[e_sorted] - r_s * RANGE
    # pads keep 0 (valid dummy local index)

    p2_edge = np.full(NCORES * G * S, -1, np.int64)
    p2_edge[p2_g] = e_sorted
    pad2 = p2_edge < 0

    # node start per core (sorted batch -> contiguous graph node ranges)
    node_start = np.searchsorted(batch, np.arange(0, B_GRAPHS, G), side="left")
    node_end = np.concatenate([node_start[1:], [batch.shape[0]]])
    assert np.all(node_end - node_start <= SLOC), "src slice exceeds SLOC"

    # bias folding (float64 for the folds, stored f32)
    b2p = (b2.astype(np.float64) - W2.astype(np.float64).sum(axis=0))
    b3p = (b3.astype(np.float64) - W3.astype(np.float64).sum(axis=0))
    b4pp = (b4.astype(np.float64) + b3p @ W4.astype(np.float64))
    b5p = (b5.astype(np.float64) - W5.astype(np.float64).sum(axis=0)).astype(
        np.float32)

    nodes_cast = np.ascontiguousarray(node_reps.astype(np_mm))
    common = {
        "nodes": nodes_cast,
        "w1c": np.ascontiguousarray(
            W1.astype(np_mm).reshape(2, 128, 4, 128).transpose(0, 2, 1, 3)),
        "w2c": np.ascontiguousarray(
            W2.astype(np_mm).reshape(4, 128, 2, 128).transpose(0, 2, 1, 3)),
        "w3c": np.ascontiguousarray(W3.astype(np_mm).reshape(2, 128, 128)),
        "w4c": np.ascontiguousarray(W4.astype(np_mm)),
        "w5c": np.ascontiguousarray(W5.astype(np_mm)),
        "b1e": b1.astype(np.float32).reshape(4, 128),
        "b1y": (b1.astype(np.float64) + 1).astype(np.float32).reshape(4, 128),
        "b2e": b2p.astype(np.float32).reshape(2, 128),
        "b2y": (b2p + 1).astype(np.float32).reshape(2, 128),
        "b4e": b4pp.astype(np.float32).reshape(1, 128),
        "b4y": (b4pp + 1).astype(np.float32).reshape(1, 128),
    }

    g_of_p2 = np.repeat(np.arange(B_GRAPHS), S)
    mbv = b5p[y[g_of_p2]].astype(np.float32)
    mbv[pad2] = -1e30
    mbv = mbv.reshape(B_GRAPHS, S)
    idxm = np.where(pad2, 0.0, p2_edge.astype(np.float64) - BIGC)
    idxm = idxm.astype(np.float32).reshape(B_GRAPHS, S)

    T = Ec // TILE
    in_maps = []
    for c in range(NCORES):
        sl = slice(c * Ec, (c + 1) * Ec)
        src_loc = srcA[sl] - node_start[c]
        src_loc = np.clip(src_loc, 0, SLOC - 1)
        gsrc16 = _wrap16(src_loc.reshape(T, TILE))
        gdst16 = _wrap16(dstL[sl].reshape(T, TILE))
        sn = np.zeros((SLOC, H), np_mm)
        hi = min(node_start[c] + SLOC, N_NODES)
        sn[: hi - node_start[c]] = nodes_cast[node_start[c]:hi]
        gg = np.arange(c * G, (c + 1) * G)
        yoff4 = np.zeros((NRANGE, G, 1), np.int32)
        for r in range(NRANGE):
            yoff4[r, :, 0] = (y[gg] * Ec + run_base[r]
                              + np.arange(G) * S_list[r]).astype(np.int32)
        m = dict(common)
        m["snodes"] = sn
        m["gsrc16"] = gsrc16
        m["gdst16"] = gdst16
        m["maskb"] = np.ascontiguousarray(mbv[c * G:(c + 1) * G])
        m["idxm"] = np.ascontiguousarray(idxm[c * G:(c + 1) * G])
        m["yoff4"] = yoff4
        in_maps.append(m)

    return in_maps, p2_edge, S_list, Ec, E


def _make_runner(nc):
    """Replicates bass2jax.run_bass_via_pjrt's multi-core path without buffer
    donation, so inputs can live on-device and the executable can be re-run
    (for timing) without re-uploading."""
    import jax
    from jax.experimental.shard_map import shard_map
    from jax.sharding import Mesh, PartitionSpec
    from concourse import bass2jax

    bass2jax.install_neuronx_cc_hook()
    partition_name = (nc.partition_id_tensor.name
                      if nc.partition_id_tensor else None)
    in_names, out_names, out_avals, zero_outs = [], [], [], []
    for alloc in nc.m.functions[0].allocations:
        if not isinstance(alloc, mybir.MemoryLocationSet):
            continue
        name = alloc.memorylocations[0].name
        if alloc.kind == "ExternalInput":
            if name != partition_name:
                in_names.append(name)
        elif alloc.kind == "ExternalOutput":
            out_names.append(name)
            sh = tuple(alloc.tensor_shape)
            dtp = mybir.dt.np(alloc.dtype)
            out_avals.append(jax.core.ShapedArray(sh, dtp))
            zero_outs.append(np.zeros(sh, dtp))
    n_params = len(in_names)
    all_in = list(in_names) + list(out_names)
    if partition_name is not None:
        all_in.append(partition_name)

    def _body(*args):
        operands = list(args)
        if partition_name is not None:
            operands.append(bass2jax.partition_id_tensor())
        outs = bass2jax._bass_exec_p.bind(
            *operands,
            out_avals=tuple(out_avals),
            in_names=tuple(all_in),
            out_names=tuple(out_names),
            lowering_input_output_aliases=(),
            sim_require_finite=True,
            sim_require_nnan=True,
            nc=nc,
        )
        return tuple(outs)

    devices = jax.devices()[:NCORES]
    mesh = Mesh(np.asarray(devices), ("core",))
    in_specs = (PartitionSpec("core"),) * (n_params + len(out_names))
    out_specs = (PartitionSpec("core"),) * len(out_names)
    fn = jax.jit(
        shard_map(_body, mesh=mesh, in_specs=in_specs, out_specs=out_specs,
                  check_rep=False),
        keep_unused=True,
    )
    return {"fn": fn, "in_names": in_names, "out_names": out_names,
            "out_avals": out_avals, "zero_outs": zero_outs, "mesh": mesh}


def _upload(runner, in_maps):
    import jax
    from jax.sharding import NamedSharding, PartitionSpec
    sh = NamedSharding(runner["mesh"], PartitionSpec("core"))
    concat_in = [
        np.concatenate([np.asarray(in_maps[c][nm]) for c in range(NCORES)],
                       axis=0)
        for nm in runner["in_names"]
    ]
    concat_zero = [
        np.zeros((NCORES * z.shape[0], *z.shape[1:]), z.dtype)
        for z in runner["zero_outs"]
    ]
    dev_in = [jax.device_put(a, sh) for a in concat_in]
    dev_zero = [jax.device_put(a, sh) for a in concat_zero]
    return dev_in, dev_zero


def _execute(runner, dev_in, dev_zero):
    import jax
    outs = runner["fn"](*dev_in, *dev_zero)
    outs = jax.block_until_ready(outs)
    res = []
    for c in range(NCORES):
        res.append({
            nm: np.asarray(outs[i]).reshape(
                NCORES, *runner["out_avals"][i].shape)[c]
            for i, nm in enumerate(runner["out_names"])
        })
    return res


def kernel(node_reps, edge_index, batch, y,
           W1, b1, W2, b2, W3, b3, W4, b4, W5, b5):
    node_reps = np.asarray(node_reps)
    in_dtype = np.asarray(edge_index).dtype

    in_maps, p2_edge, S_list, Ec, E = _host_prep(
        node_reps, edge_index, batch, y,
        np.asarray(W1), np.asarray(b1), np.asarray(W2), np.asarray(b2),
        np.asarray(W3), np.asarray(b3), np.asarray(W4), np.asarray(b4),
        np.asarray(W5), np.asarray(b5))

    key = (Ec, tuple(S_list), "f32")
    if key not in _NC_CACHE:
        nc = _build_nc(Ec, S_list, F32)
        _NC_CACHE[key] = {"nc": nc, "runner": _make_runner(nc)}
    runner = _NC_CACHE[key]["runner"]

    dev_in, dev_zero = _upload(runner, in_maps)
    results = _execute(runner, dev_in, dev_zero)
    _LAST.update(runner=runner, dev_in=dev_in, dev_zero=dev_zero)

    probs_all = np.concatenate(
        [results[c]["probs"].ravel() for c in range(NCORES)])
    segmax = np.concatenate(
        [results[c]["segmax"].ravel() for c in range(NCORES)])
    act = np.concatenate(
        [results[c]["act"].ravel() for c in range(NCORES)])

    probs_full = np.zeros(E, np.float32)
    valid = p2_edge >= 0
    probs_full[p2_edge[valid]] = probs_all[valid]
    actions = np.rint(act).astype(in_dtype if in_dtype in (np.int32, np.int64)
                                  else np.int32)
    return probs_full, segmax.astype(np.float32), actions
